# revision 177
# baseline (speedup 1.0000x reference)
"""Causal multi-head attention block (qkv proj + RoPE + RMSNorm + SDPA +
out proj) on 8 TRN2 NeuronCores.

Sharding: core c handles batch b = c//2 and head-group g = c%2 (8 of 16
heads).  Each core computes its batch's qkv projection for its heads,
full causal attention, and a *partial* output projection over its 512
channels for all 2048 rows of its batch.  The host sums the two partial
outputs per batch (bf16 partials, f32 sum) — no on-device collectives.

Attention runs in a transposed ("scoresT") layout: scoresT[k, q] =
k_blk @ q_chunk^T so softmax needs no transposes of p; the softmax
denominator comes free from a ones-column appended to V; there is no
max-subtraction (rms-normalized q, k bound |score| <= 8 so exp never
overflows).

Performance notes (tuned against the InstructionCostModel timeline):
- all matmul operands are bf16: fp32r pays 4x for moving dims < 256
  (the causal-diagonal blocks), bf16 is 1 cycle/row at any width;
- causal masking multiplies exp output by a 0/1 wedge on the DVE
  instead of adding -1e30 via an extra PE matmul;
- the AV-accumulate queue is global across head-pairs, so the next
  pair's QK matmuls (which feed the Exp-bound Activation engine) issue
  ahead of the previous pair's trailing AV matmuls;
- out-projection matmuls of chunk c-1 and the 1/denominator
  normalization (ones-matmul broadcast + in-place multiply, deferred
  by one head-pair) fill the PE while each pair's softmax denominator
  chain drains;
- the last two row-blocks' qkv psum is staged through SBUF with one
  copy, and the last four transpose batches are deferred into the
  attention stream, to shorten the phase-1 -> phase-2 psum handover.
"""
import sys

for _p in ("/root/.axon_site/_ro/trn_rl_repo", "/opt/trn_rl_repo"):
    if _p not in sys.path:
        sys.path.append(_p)

import numpy as np

import concourse.bass as bass
import concourse.mybir as mybir
import concourse.tile as tile
from concourse.alu_op_type import AluOpType
from concourse.bass_utils import run_bass_kernel_spmd
from concourse.vector_clock import ScopedClock

# ---------------------------------------------------------------------------
# Patch TileContext._drain_and_barrier: this container's walrus rejects the
# stock exit path (multi-wait Drain + butterfly-barrier Drains with sem-eq
# waits) with "Too many sync wait commands".  Carry the exit waits one per
# NOP ahead of a bare drain, and use the sem-only EVSEM barrier.
# ---------------------------------------------------------------------------


def _drain_and_barrier(self, tick_clock, wait_clock):
    probe = self.nc.sync.nop(nofuse=True, hint="tile_exit_wait_probe")
    wait_clock.add_sem_waits(
        probe.ins, ScopedClock({None: tick_clock.global_clock})
    )
    waits = list(probe.ins.sync_info.on_wait) if probe.ins.sync_info else []
    if len(waits) > 1:
        probe.ins.sync_info.on_wait = waits[:1]
        for w in waits[1:]:
            carrier = self.nc.sync.nop(nofuse=True, hint="tile_exit_wait")
            carrier.ins.sync_info = mybir.SyncInfo(on_wait=[w], on_update=[])
    self.nc.sync.drain()

    self.nc.all_engine_barrier(sem_only=True)
    assert self.sems is not None
    popped = self.nc._tile_sem_poison_stack.pop()
    assert popped is self._sem_poison
    self.nc.clear_and_free_semaphores(list(self.sems.allocated().values()))
    self.nc.all_engine_barrier(sem_only=True)


tile.TileContext._drain_and_barrier = _drain_and_barrier

_MAXW = 1
_nop_ctr = [0]


def _split_waits(nc):
    """Hoist excess sem waits onto single-wait NOPs ahead of each
    instruction — this walrus's codegen allows very few sync-wait
    commands per instruction struct."""
    for fn in nc.m.functions:
        for blk in fn.blocks:
            out = []
            for inst in blk.instructions:
                si = inst.sync_info
                waits = list(si.on_wait) if si and si.on_wait else []
                if len(waits) > _MAXW:
                    for w in waits[:-_MAXW]:
                        _nop_ctr[0] += 1
                        out.append(mybir.InstNoOp(
                            name=f"wsplit-{_nop_ctr[0]}",
                            engine=inst.engine,
                            bass_nofuse=True,
                            sync_info=mybir.SyncInfo(on_wait=[w], on_update=[]),
                        ))
                    si.on_wait = waits[-_MAXW:]
                out.append(inst)
            blk.instructions = out

# ---------------------------------------------------------------------------

B, T, C = 4, 2048, 1024
H, D = 16, 64
G = 2            # head groups (one per core within a batch pair)
HG = H // G      # 8 heads per core
NP = HG // 2     # 4 head pairs per core
TB = T // 128    # 16 row blocks
CT = C // 128    # 8 contraction tiles
NCH = T // 512   # 4 q chunks
EPS = 1e-6
SCALE = 1.0 / float(np.sqrt(D))

F32 = mybir.dt.float32
F32R = mybir.dt.float32r
BF16 = mybir.dt.bfloat16
import os as _os
MM_DT = _os.environ.get("KMM_DT", "bf16")
MD = {"f32r": F32R, "f32": F32, "bf16": BF16}[MM_DT]
AX = mybir.AxisListType
AF = mybir.ActivationFunctionType


def _bc_mid(ap2d, n):
    """[P, X] -> [P, n, X] with broadcast (step 0) middle dim."""
    return bass.AP(tensor=ap2d.tensor, offset=ap2d.offset,
                   ap=[ap2d.ap[0], [0, n], ap2d.ap[1]])


def _bc_last(ap2d, n):
    """[P, X] -> [P, X, n] with broadcast (step 0) last dim."""
    return bass.AP(tensor=ap2d.tensor, offset=ap2d.offset,
                   ap=[ap2d.ap[0], ap2d.ap[1], [0, n]])


def _bc_mid2(ap2d, n1, n2):
    """[P, X] -> [P, n1, n2, X] with two broadcast middle dims."""
    return bass.AP(tensor=ap2d.tensor, offset=ap2d.offset,
                   ap=[ap2d.ap[0], [0, n1], [0, n2], ap2d.ap[1]])


def _bc_part(ap1d, n):
    """[1, X] -> [n, X] partition-broadcast (step 0) AP."""
    return bass.AP(tensor=ap1d.tensor, offset=ap1d.offset,
                   ap=[[0, n], ap1d.ap[1]])


def _flush_tp(nc, pool, iden_sb, item, aux=False):
    qf, dstT, tb = item
    qf_f = qf.rearrange("p h d -> p (h d)")
    for hp in range(NP):
        if aux:
            tp = pool.tile([128, 1024], MD, tag="aux", name="tpa")[:, 0:128]
        else:
            tp = pool.tile([128, 128], MD, tag="tp")
        nc.tensor.transpose(tp, qf_f[:, hp * 128:(hp + 1) * 128], iden_sb[:, :])
        dst = dstT[tb // 4][:, hp, (tb % 4) * 128:(tb % 4 + 1) * 128]
        if aux:
            nc.vector.tensor_copy(out=dst, in_=tp)
        else:
            nc.scalar.copy(out=dst, in_=tp)


def _emit_av(nc, av, v_sb, hp, j, off, pt, nj):
    for h in range(2):
        nc.tensor.matmul(
            av[h][:, off:512],
            lhsT=v_sb[j // 4][:, j % 4, 2 * hp + h, :],
            rhs=pt[:, h, off:512],
            start=(j == 0), stop=(j == nj - 1),
            skip_group_check=True,
        )


def _emit_norm(nc, p23x, ones_sb, item):
    yT_ref, hp, recs = item
    for h in range(2):
        rps = p23x.tile([128, 512], F32, tag="aux", name="rps")
        nc.tensor.matmul(
            rps[0:64, :],
            lhsT=ones_sb[:, :],
            rhs=recs[h],
            start=True, stop=True,
            skip_group_check=True,
        )
        nc.vector.tensor_tensor(yT_ref[64 * h:64 * (h + 1), hp, :],
                                yT_ref[64 * h:64 * (h + 1), hp, :],
                                rps[0:64, :],
                                op=AluOpType.mult)


def _emit_po(nc, p23x, p3, wpT_sb, out, yT_prev, cp, tt, oc):
    ts_ = slice(tt * 128, (tt + 1) * 128)
    tl = (tt % 4) * 128
    po = p23x.tile([128, 512], F32, tag="aux", name="po")
    for ct in range(4):
        nc.tensor.matmul(
            po,
            lhsT=yT_prev[:, ct, tl:tl + 128],
            rhs=wpT_sb[:, ct, oc * 512:(oc + 1) * 512],
            start=(ct == 0), stop=(ct == 3),
        )
    ost = p3.tile([128, 512], MD, tag="ost")
    nc.vector.tensor_copy(out=ost, in_=po)
    nc.sync.dma_start(out=out[ts_, oc * 512:(oc + 1) * 512], in_=ost)


def build_bass():
    nc = bass.Bass("TRN2")

    xTt = nc.declare_dram_parameter("xTt", [TB, CT, 128, 128], MD, isOutput=False)
    wTt = nc.declare_dram_parameter("wTt", [CT, 128, 3 * 512], MD, isOutput=False)
    wpTt = nc.declare_dram_parameter("wpTt", [4, 128, C], MD, isOutput=False)
    cost = nc.declare_dram_parameter("cost", [TB, 128, 32], F32, isOutput=False)
    sint = nc.declare_dram_parameter("sint", [TB, 128, 32], F32, isOutput=False)
    wedge = nc.declare_dram_parameter("wedge", [128, 128], MD, isOutput=False)
    iden = nc.declare_dram_parameter("iden", [128, 128], MD, isOutput=False)
    out = nc.declare_dram_parameter("out", [T, C], MD, isOutput=True)

    with tile.TileContext(nc) as tc:
        with tc.tile_pool(name="res", bufs=1) as res:
            wedge_sb = res.tile([128, 128], MD)
            iden_sb = res.tile([128, 128], MD)
            ones_sb = res.tile([1, 64], F32R)
            nc.vector.memset(ones_sb.bitcast(F32), 1.0)

            qT_sb = [res.tile([128, NP, 512], MD, name=f"qT{i}")
                     for i in range(NCH)]   # per-chunk [h*64+d, pair, 512]
            kT_sb = [res.tile([128, NP, 512], MD, name=f"kT{i}")
                     for i in range(NCH)]
            v_sb = [res.tile([128, 4, HG, 65], MD, name=f"v{i}")
                    for i in range(NCH)]
            for i in range(NCH):
                if MD == F32R:
                    nc.vector.memset(v_sb[i][:, :, :, 64:65].bitcast(F32), 1.0)
                else:
                    nc.vector.memset(v_sb[i][:, :, :, 64:65], 1.0)

            # ---------------- Phase 1: qkv + rope + rms + transpose --------
            with (
                tc.tile_pool(name="p1r", bufs=1) as p1r,
                tc.tile_pool(name="p1", bufs=4) as p1,
                tc.tile_pool(name="p1s", bufs=4) as p1s,
                tc.tile_pool(name="p1ps", bufs=2, space="PSUM") as p1ps,
                tc.tile_pool(name="p1tp", bufs=2, space="PSUM") as p1tp,
            ):
                wT_sb = p1r.tile([128, CT, 3 * 512], MD)
                nc.sync.dma_start(out=wT_sb[:, 0, :], in_=wTt[0])
                for _ct in range(1, CT):
                    eng = nc.scalar if _ct % 2 else nc.gpsimd
                    eng.dma_start(out=wT_sb[:, _ct, :], in_=wTt[_ct])
                cos_sb = p1r.tile([128, TB, 32], F32)
                nc.scalar.dma_start(out=cos_sb, in_=cost.rearrange("b p d -> p b d"))
                sin_sb = p1r.tile([128, TB, 32], F32)
                nc.scalar.dma_start(out=sin_sb, in_=sint.rearrange("b p d -> p b d"))
                eps_sb = p1r.tile([128, 1], F32)
                nc.vector.memset(eps_sb, EPS)
                nc.scalar.dma_start(out=iden_sb, in_=iden[:, :])
                nc.scalar.dma_start(out=wedge_sb, in_=wedge[:, :])
                pend_tp = []
                for tb in range(TB):
                    xt = p1.tile([128, CT, 128], MD, tag="xt", bufs=6)
                    if tb == 0:
                        nc.sync.dma_start(
                            out=xt[:, 0:2, :],
                            in_=xTt[tb, 0:2].rearrange("c p t -> p c t"))
                        nc.sync.dma_start(
                            out=xt[:, 2:CT, :],
                            in_=xTt[tb, 2:CT].rearrange("c p t -> p c t"))
                    else:
                        nc.sync.dma_start(
                            out=xt, in_=xTt[tb].rearrange("c p t -> p c t"))
                    ps = {}
                    for qn in ("q", "k", "v"):
                        ps[qn] = p1ps.tile([128, 512], F32, tag=f"ps{qn}",
                                           name=f"ps{qn}")
                    if tb < 2:
                        # ct-major interleave: every arriving weight chunk
                        # unlocks three matmuls while the DMA ingest runs
                        for ct in range(CT):
                            for qi, qn in enumerate(("q", "k", "v")):
                                nc.tensor.matmul(
                                    ps[qn],
                                    lhsT=xt[:, ct, :],
                                    rhs=wT_sb[:, ct, qi * 512:(qi + 1) * 512],
                                    start=(ct == 0), stop=(ct == CT - 1),
                                )
                    else:
                        for qi, qn in enumerate(("q", "k", "v")):
                            for ct in range(CT):
                                nc.tensor.matmul(
                                    ps[qn],
                                    lhsT=xt[:, ct, :],
                                    rhs=wT_sb[:, ct, qi * 512:(qi + 1) * 512],
                                    start=(ct == 0), stop=(ct == CT - 1),
                                )

                    # cos/sin broadcast over heads and over the two rope halves
                    cosb = _bc_mid2(cos_sb[:, tb, :], HG, 2)
                    sinb = _bc_mid2(sin_sb[:, tb, :], HG, 2)
                    srcs_map = {}
                    if tb >= TB - 2 or tb < 2:
                        # stage the last tiles through SBUF with a single
                        # copy (emitted before any rope work) so the psum
                        # banks free quickly for attention (phase-boundary
                        # WAR)
                        for name in ("q", "k"):
                            s4 = ps[name].rearrange("p (h u d) -> p h u d",
                                                    u=2, d=32)
                            srcs = p1s.tile([128, HG, 2, 32], F32, tag="srcs", bufs=4)
                            if tb < 2:
                                nc.scalar.copy(out=srcs, in_=s4)
                            else:
                                nc.vector.tensor_copy(out=srcs, in_=s4)
                            srcs_map[name] = srcs
                    for name, dstT in (("q", qT_sb), ("k", kT_sb)):
                        src4 = ps[name].rearrange("p (h u d) -> p h u d", u=2, d=32)
                        if name in srcs_map:
                            src4 = srcs_map[name]
                        ca = p1s.tile([128, HG, 2, 32], F32, tag="ca")
                        cb = p1s.tile([128, HG, 2, 32], F32, tag="cb")
                        nc.vector.tensor_tensor(ca, src4, cosb, op=AluOpType.mult)
                        nc.vector.tensor_tensor(cb, src4, sinb, op=AluOpType.mult)
                        ro = p1.tile([128, HG, 2, 32], F32, tag="ro", bufs=4)
                        nc.gpsimd.tensor_tensor(ro[:, :, 0, :], ca[:, :, 0, :],
                                                cb[:, :, 1, :], op=AluOpType.add)
                        nc.gpsimd.tensor_tensor(ro[:, :, 1, :], ca[:, :, 1, :],
                                                cb[:, :, 0, :], op=AluOpType.subtract)
                        ro3 = ro.rearrange("p h u d -> p h (u d)")
                        sq = p1s.tile([128, HG, 64], F32, tag="sq")
                        nc.scalar.activation(out=sq, in_=src4.rearrange(
                            "p h u d -> p h (u d)"), func=AF.Square)
                        ss = p1.tile([128, HG], F32, tag="ss", bufs=6)
                        nc.vector.reduce_sum(out=ss, in_=sq, axis=AX.X)
                        sd = p1.tile([128, HG], F32, tag="sd", bufs=6)
                        nc.scalar.activation(out=sd, in_=ss, func=AF.Sqrt,
                                             bias=eps_sb[:, 0:1], scale=1.0 / 64.0)
                        rs = p1.tile([128, HG], F32, tag="rs", bufs=6)
                        nc.vector.reciprocal(out=rs, in_=sd)
                        qf = p1.tile([128, HG, 64], MD, tag="qf", bufs=8)
                        nc.gpsimd.tensor_tensor(qf, ro3, _bc_last(rs, 64),
                                                op=AluOpType.mult)
                        pend_tp.append((qf, dstT, tb))
                    nc.scalar.copy(out=v_sb[tb // 4][:, tb % 4, :, 0:64],
                                   in_=ps["v"].rearrange("p (h d) -> p h d", d=64))
                    while len(pend_tp) > 2:
                        _flush_tp(nc, p1tp, iden_sb, pend_tp.pop(0))
                while len(pend_tp) > 8:
                    _flush_tp(nc, p1tp, iden_sb, pend_tp.pop(0))
                # last 4 (tb14/15 q,k) are deferred into the attention
                # stream so the PE is not stuck behind their rope chains

            # ------------- Phase 2+3: attention + partial out proj ---------
            with (
                tc.tile_pool(name="p23r", bufs=1) as p23r,
                tc.tile_pool(name="p23y", bufs=4) as p23y,
                tc.tile_pool(name="p2", bufs=4) as p2,
                tc.tile_pool(name="p3", bufs=8) as p3,
                tc.tile_pool(name="p2s", bufs=2, space="PSUM") as p2s,
                tc.tile_pool(name="p2av", bufs=1, space="PSUM") as p2av,
                tc.tile_pool(name="p23x", bufs=2, space="PSUM") as p23x,
            ):
                wpT_sb = p23r.tile([128, 4, C], MD)
                for _w in range(4):
                    _weng = [nc.gpsimd, nc.scalar, nc.gpsimd, nc.scalar][_w]
                    _weng.dma_start(
                        out=wpT_sb[:, _w:_w + 1, :],
                        in_=wpTt[_w:_w + 1].rearrange("c p n -> p c n"))
                pend_po = []
                pend_norm = []
                pend_av = []    # global across hp: (avs, hp, j, off, pt, nj, yT)

                def pop_av():
                    avs, hp_, j_, off_, pt_, nj_, yT_ = pend_av.pop(0)
                    if avs[0] is None:
                        avs[0] = p2av.tile([65, 512], F32, tag="av0",
                                           name="av0")
                        avs[1] = p2av.tile([65, 512], F32, tag="av1",
                                           name="av1")
                    _emit_av(nc, avs, v_sb, hp_, j_, off_, pt_, nj_)
                    if j_ == nj_ - 1:
                        recs = []
                        for h in range(2):
                            rec = p2.tile([1, 512], F32R, tag="rec", bufs=6)
                            with nc.allow_low_precision(reason="f32r recip"):
                                nc.vector.reciprocal(out=rec,
                                                     in_=avs[h][64:65, :])
                            nc.vector.tensor_copy(
                                out=yT_[h * 64:(h + 1) * 64, hp_, :],
                                in_=avs[h][0:64, :])
                            recs.append(rec)
                        pend_norm.append((yT_, hp_, recs))

                def drain_norms(yT):
                    i = 0
                    while i < len(pend_norm):
                        if pend_norm[i][0] is yT:
                            _emit_norm(nc, p23x, ones_sb, pend_norm.pop(i))
                        else:
                            i += 1

                for c in range(NCH):
                    yT_c = p23y.tile([128, NP, 512], MD, tag="yTc")
                    for hp in range(NP):
                        for _ in range(2):
                            if pend_norm:
                                _emit_norm(nc, p23x, ones_sb,
                                           pend_norm.pop(0))
                        if pend_tp:
                            _flush_tp(nc, p23x, iden_sb, pend_tp.pop(0),
                                      aux=True)
                        avs = [None, None]
                        nj = 4 * c + 4
                        for j in range(nj):
                            off = max(128 * j - 512 * c, 0)
                            sps = p2s.tile([128, 2, 512], F32, tag="sps")
                            diag = 128 * j - 512 * c >= 0
                            for h in range(2):
                                nc.tensor.matmul(
                                    sps[:, h, off:512],
                                    lhsT=kT_sb[j // 4][h * 64:(h + 1) * 64, hp,
                                                       (j % 4) * 128:(j % 4 + 1) * 128],
                                    rhs=qT_sb[c][h * 64:(h + 1) * 64, hp, off:512],
                                    start=True, stop=True,
                                    skip_group_check=True,
                                )
                            pt = p2.tile([128, 2, 512], MD, tag="pt", bufs=10)
                            nc.scalar.activation(out=pt[:, :, off:512],
                                                 in_=sps[:, :, off:512],
                                                 func=AF.Exp, scale=SCALE)
                            if diag:
                                nc.vector.tensor_tensor(
                                    pt[:, :, off:off + 128],
                                    pt[:, :, off:off + 128],
                                    _bc_mid(wedge_sb, 2),
                                    op=AluOpType.mult)
                            pend_av.append((avs, hp, j, off, pt, nj, yT_c))
                            if len(pend_av) > 8:
                                pop_av()
                        nfill = 1 if c == NCH - 1 else 2
                        for _ in range(nfill):
                            if pend_po:
                                item = pend_po[0]
                                drain_norms(item[0])
                                _emit_po(nc, p23x, p3, wpT_sb, out,
                                         *pend_po.pop(0))
                    po_left = pend_po
                    pend_po = [(yT_c, c, tt, oc)
                               for tt in range(4 * c, 4 * c + 4)
                               for oc in range(2)]
                while pend_av:
                    pop_av()
                    if po_left and len(pend_av) % 2 == 0:
                        item = po_left[0]
                        drain_norms(item[0])
                        _emit_po(nc, p23x, p3, wpT_sb, out, *po_left.pop(0))
                while po_left or pend_norm:
                    if pend_norm:
                        _emit_norm(nc, p23x, ones_sb, pend_norm.pop(0))
                    if po_left:
                        _emit_po(nc, p23x, p3, wpT_sb, out, *po_left.pop(0))
                for item in pend_po:
                    _emit_po(nc, p23x, p3, wpT_sb, out, *item)
    _split_waits(nc)
    return nc


if MD == BF16:
    import ml_dtypes as _mld
    MD_NP = _mld.bfloat16
else:
    MD_NP = np.float32


def prep_core_inputs(x, w_attn, w_proj, cos, sin, core):
    b, g = core // 2, core % 2
    xT = np.ascontiguousarray(x[b].T)                       # [C, T]
    xTt = np.ascontiguousarray(
        xT.reshape(CT, 128, TB, 128).transpose(2, 0, 1, 3)).astype(MD_NP)
    qr = np.arange(g * 512, g * 512 + 512)
    rows = np.concatenate([qr, C + qr, 2 * C + qr])
    wT = np.ascontiguousarray(w_attn[rows, :].T)            # [C, 1536]
    wTt = np.ascontiguousarray(wT.reshape(CT, 128, 3 * 512)).astype(MD_NP)
    wpT = np.ascontiguousarray(w_proj.T[g * 512:(g + 1) * 512, :])  # [512, C]
    wpTt = np.ascontiguousarray(wpT.reshape(4, 128, C)).astype(MD_NP)
    cost = np.ascontiguousarray(cos.reshape(TB, 128, 32))
    sint = np.ascontiguousarray(sin.reshape(TB, 128, 32))
    kl = np.arange(128, dtype=np.float32)[:, None]
    ql = np.arange(128, dtype=np.float32)[None, :]
    wedge = np.where(ql >= kl, 1.0, 0.0).astype(MD_NP)
    iden = np.eye(128, dtype=np.float32).astype(MD_NP)
    return dict(xTt=xTt, wTt=wTt, wpTt=wpTt, cost=cost, sint=sint,
                wedge=wedge, iden=iden)


_CACHED_NC = None


def kernel(x, cos, sin, w_attn, w_proj, _want_results=False, **_ignored):
    global _CACHED_NC
    x = np.ascontiguousarray(np.asarray(x, dtype=np.float32))
    w_attn = np.ascontiguousarray(np.asarray(w_attn, dtype=np.float32))
    w_proj = np.ascontiguousarray(np.asarray(w_proj, dtype=np.float32))
    cosn = np.ascontiguousarray(np.asarray(cos, dtype=np.float32)[0, :, 0, :])
    sinn = np.ascontiguousarray(np.asarray(sin, dtype=np.float32)[0, :, 0, :])

    if _CACHED_NC is None:
        _CACHED_NC = build_bass()
    nc = _CACHED_NC

    in_maps = [prep_core_inputs(x, w_attn, w_proj, cosn, sinn, c)
               for c in range(8)]
    res = run_bass_kernel_spmd(nc, in_maps, core_ids=list(range(8)))

    out = np.zeros((B, T, C), np.float32)
    for b in range(B):
        out[b] = (res.results[2 * b]["out"].astype(np.float32)
                  + res.results[2 * b + 1]["out"].astype(np.float32))
    if _want_results:
        return out, res
    return out

